# revision 24
# baseline (speedup 1.0000x reference)
"""Trainium2 Bass kernel for nn_CrossAttention (b=512, n_q=1, n_kv=512, dim=256,
heads=8, dim_head=64, topk=4).

Strategy
--------
Pure data parallel: 64 instances per NeuronCore x 8 cores. Since n_q == 1, the
K/V projections are folded algebraically so the device never materializes K or V:

    dots[i,h,j] = sum_d t[i,h,d] * kv[i,j,d]      t = (q @ Wq)*scale  folded with Wk (host, tiny)
    u[i,h,d]    = sum_j attn[i,h,j] * kv[i,j,d]
    out[i,:]    = sum_{h,d} u[i,h,d] * W2[h,d,:] + bo,   W2[h] = Wv_h @ Wo_h (host, tiny)

This reduces device FLOPs ~40x and makes the kernel HBM-bound on streaming
inp_kv. kv is shipped in BOTH orientations (j-major for the attn@kv contraction,
d-major for the scores contraction) because the PE contracts over the partition
dim of both operands; fp32 is kept end-to-end because the top-k indices of
head_sum have relative margins down to 7.6e-5 (bf16 provably flips them).

Layout: 4 instances per 128-partition "block" (col-tiled matmuls, 32-partition
stride, heads padded 8->32 with zeros), 16 blocks per core, grouped in 4 quads
of 16 instances for head_sum/top-k and the output projection.
"""
import os
import sys

sys.path.insert(0, "/opt/trn_rl_repo")

import numpy as np

from concourse import bacc, bass, mybir
from concourse.tile import TileContext
from concourse.bass_utils import run_bass_kernel_spmd

F32 = mybir.dt.float32
F32R = mybir.dt.float32r
U32 = mybir.dt.uint32
AF = mybir.ActivationFunctionType

B, NCORES = 512, 8
BL = B // NCORES          # 64 instances per core
H, DH, D, J, E = 8, 64, 256, 512, 256
NBLK = BL // 4            # 16 blocks of 4 instances
NQUAD = NBLK // 4         # 4 quads of 16 instances

_CACHED = {}
LAST_RESULT = None


def _build_nc(use_f32r_dots: bool):
    nc = bacc.Bacc()
    tq_d = nc.declare_dram_parameter("tq", [128, NBLK * 2 * 4 * 32], F32, isOutput=False)
    w2_d = nc.declare_dram_parameter("w2", [128, H * 2 * E], F32, isOutput=False)
    neg_d = nc.declare_dram_parameter("neg", [4, NBLK * J], F32, isOutput=False)
    negind_d = nc.declare_dram_parameter("negind", [4, 128], F32, isOutput=False)
    hsm_d = nc.declare_dram_parameter("hsm", [128, 4 * 16], F32, isOutput=False)
    ident_d = nc.declare_dram_parameter("ident", [128, 128], F32, isOutput=False)
    onesbo_d = nc.declare_dram_parameter("onesbo", [1, 16 + E], F32, isOutput=False)
    kv_d = nc.declare_dram_parameter("kv", [NBLK, 128, 4 * 4 * D], F32, isOutput=False)
    kvt_d = nc.declare_dram_parameter("kvt", [NBLK, 128, 4 * 2 * J], F32, isOutput=False)
    out_d = nc.declare_dram_parameter("out", [BL, E], F32, isOutput=True)
    idx_d = nc.declare_dram_parameter("idx", [BL, 8], U32, isOutput=True)

    # fp32r needs inputs rounded-to-fp32r at the producer (walrus birverifier);
    # V1 runs every matmul in plain fp32 for exactness.
    def mmcast(ap):
        return ap

    dots_cast = mmcast

    with TileContext(nc) as tc:
        with tc.tile_pool(name="const", bufs=1) as cpool, \
             tc.tile_pool(name="kv", bufs=2) as kvpool, \
             tc.tile_pool(name="work", bufs=2) as wpool, \
             tc.tile_pool(name="grp", bufs=2) as gpool, \
             tc.tile_pool(name="dots_ps", bufs=2, space="PSUM") as dots_ps, \
             tc.tile_pool(name="att_ps", bufs=1, space="PSUM") as att_ps, \
             tc.tile_pool(name="u_ps", bufs=1, space="PSUM") as u_ps, \
             tc.tile_pool(name="hs_ps", bufs=1, space="PSUM") as hs_ps, \
             tc.tile_pool(name="fin_ps", bufs=2, space="PSUM") as fin_ps, \
             tc.tile_pool(name="out_ps", bufs=1, space="PSUM") as out_ps:

            tq_sb = cpool.tile([128, NBLK * 2 * 4 * 32], F32)
            w2_sb = cpool.tile([128, H * 2 * E], F32)
            neg_sb = cpool.tile([4, NBLK * J], F32)
            negind_sb = cpool.tile([4, 128], F32)
            hsm_sb = cpool.tile([128, 4 * 16], F32)
            ident_sb = cpool.tile([128, 128], F32)
            onesbo_sb = cpool.tile([1, 16 + E], F32)
            nc.sync.dma_start(tq_sb[:, :], tq_d[:, :])
            nc.sync.dma_start(w2_sb[:, :], w2_d[:, :])
            nc.sync.dma_start(neg_sb[:, :], neg_d[:, :])
            nc.sync.dma_start(negind_sb[:, :], negind_d[:, :])
            nc.sync.dma_start(hsm_sb[:, :], hsm_d[:, :])
            nc.sync.dma_start(ident_sb[:, :], ident_d[:, :])
            nc.sync.dma_start(onesbo_sb[:, :], onesbo_d[:, :])

            for q in range(NQUAD):
                hs_psum = hs_ps.tile([16, J], F32, tag="hs")
                # U^T staging for the whole quad: [128 d, dh, m*8+h] (m = quad-local instance)
                ut_sb = gpool.tile([128, 2, 128], F32, tag="ut_sb")

                for pi in range(4):          # block within quad
                    b = q * 4 + pi
                    kvt_blk = kvpool.tile([128, 4 * 2 * J], F32, tag="kvt")
                    kv_blk = kvpool.tile([128, 4 * 4 * D], F32, tag="kv")
                    nc.sync.dma_start(kvt_blk[:, :], kvt_d[b, :, :])
                    nc.sync.dma_start(kv_blk[:, :], kv_d[b, :, :])
                    kvt_g = [kvt_blk[:, 2 * J * g:2 * J * (g + 1)] for g in range(4)]
                    kv_g = [kv_blk[:, 4 * D * g:4 * D * (g + 1)] for g in range(4)]

                    # scores: dots[(g,h), j] = sum_d t^T[d, (g,h)] * kvT[d, j]
                    dots_psum = dots_ps.tile([128, J], F32, tag="dots")
                    for dh in range(2):
                        for g in range(4):
                            col = ((b * 2 + dh) * 4 + g) * 32
                            nc.tensor.matmul(
                                dots_psum[32 * g:32 * g + 32, :],
                                dots_cast(tq_sb[:, col:col + 32]),
                                dots_cast(kvt_g[g][:, dh * J:(dh + 1) * J]),
                                start=(dh == 0), stop=False,
                                tile_position=(0, 32 * g),
                                skip_group_check=True,
                            )
                    # mask add: + 1[g broadcast] @ neg rows of this block
                    nc.tensor.matmul(
                        dots_psum[:, :],
                        mmcast(negind_sb[:, :]),
                        mmcast(neg_sb[:, b * J:(b + 1) * J]),
                        start=False, stop=True,
                        skip_group_check=True,
                    )

                    # exp + per-row partition sums (softmax denominator)
                    exp_sb = wpool.tile([128, J], F32, tag="exp")
                    z_sb = wpool.tile([128, 1], F32, tag="z")
                    rz_sb = wpool.tile([128, 1], F32, tag="rz")
                    nc.scalar.activation(exp_sb[:, :], dots_psum[:, :], AF.Exp,
                                         accum_out=z_sb[:, :])
                    nc.vector.reciprocal(rz_sb[:, :], z_sb[:, :])

                    # head_sum (normalized) accumulated across the quad:
                    # lhsT = rz * block-diagonal head mask
                    hsind_sb = wpool.tile([128, 16], F32, tag="hsind")
                    nc.vector.tensor_scalar_mul(
                        hsind_sb[:, :], hsm_sb[:, 16 * pi:16 * pi + 16], rz_sb[:, 0:1])
                    nc.tensor.matmul(
                        hs_psum[:, :], mmcast(hsind_sb[:, :]), mmcast(exp_sb[:, :]),
                        start=(pi == 0), stop=(pi == 3),
                        skip_group_check=True,
                    )

                    # attn^T via PE transpose (exact, unnormalized exp)
                    att_psum = att_ps.tile([128, J], F32, tag="attT")
                    for jc in range(4):
                        nc.tensor.transpose(
                            att_psum[:, 128 * jc:128 * jc + 128],
                            exp_sb[:, 128 * jc:128 * jc + 128],
                            ident_sb[:, :])
                    attT_sb = wpool.tile([128, J], F32, tag="attT_sb")
                    nc.vector.tensor_copy(attT_sb[:, 0:256], att_psum[:, 0:256])
                    nc.scalar.activation(attT_sb[:, 256:512], att_psum[:, 256:512], AF.Copy)

                    # u[(g,h), d] = sum_j attn^T[j, (g,h)] * kv[j, d]
                    u_psum = u_ps.tile([128, D], F32, tag="u")
                    for jc in range(4):
                        for g in range(4):
                            nc.tensor.matmul(
                                u_psum[32 * g:32 * g + 32, :],
                                mmcast(attT_sb[:, 128 * jc + 32 * g:128 * jc + 32 * g + 32]),
                                mmcast(kv_g[g][:, D * jc:D * (jc + 1)]),
                                start=(jc == 0), stop=(jc == 3),
                                tile_position=(0, 32 * g),
                                skip_group_check=True,
                            )

                    # normalize u by 1/Z (full-width; pad rows harmless), then
                    # transpose so the 32-stride instance layout lands on the
                    # free dim where strided slicing is legal
                    u_norm = wpool.tile([128, D], F32, tag="u_norm")
                    nc.vector.tensor_scalar_mul(
                        u_norm[:, 0:128], u_psum[:, 0:128], rz_sb[:, 0:1])
                    nc.scalar.activation(
                        u_norm[:, 128:256], u_psum[:, 128:256], AF.Copy,
                        scale=rz_sb[:, 0:1])
                    ut_psum = fin_ps.tile([128, D], F32, tag="ut")
                    for dhf in range(2):
                        nc.tensor.transpose(
                            ut_psum[:, 128 * dhf:128 * dhf + 128],
                            u_norm[:, 128 * dhf:128 * dhf + 128],
                            ident_sb[:, :])
                    for dhf in range(2):
                        src = (ut_psum[:, 128 * dhf:128 * dhf + 128]
                               .rearrange("p (g r) -> p g r", g=4)[:, :, 0:8])
                        dst = (ut_sb[:, dhf:dhf + 1, 32 * pi:32 * pi + 32]
                               .rearrange("p x (g r) -> p (x g) r", g=4))
                        if dhf == 0:
                            nc.vector.tensor_copy(dst, src)
                        else:
                            nc.scalar.activation(dst, src, AF.Copy)

                # ---- per quad: top-k of head_sum ----
                hs_sb = gpool.tile([16, J], F32, tag="hs_sb")
                nc.vector.tensor_copy(hs_sb[:, :], hs_psum[:, :])
                hsv_sb = gpool.tile([16, 8], F32, tag="hsv")
                hsi_sb = gpool.tile([16, 8], U32, tag="hsi")
                nc.vector.max(hsv_sb[:, :], hs_sb[:, :])
                nc.vector.max_index(hsi_sb[:, :], hsv_sb[:, :], hs_sb[:, :])
                nc.sync.dma_start(idx_d[16 * q:16 * q + 16, :], hsi_sb[:, :])

                # ---- per quad: out = U @ W2 + bo ----
                out_psum = out_ps.tile([16, E], F32, tag="outp")
                for dhf in range(2):
                    for h in range(H):
                        # 16 columns m*8+h, single stride 8
                        lhs = ut_sb[:, dhf, h:h + 121:8]
                        rhs = w2_sb[:, (h * 2 + dhf) * E:(h * 2 + dhf + 1) * E]
                        nc.tensor.matmul(
                            out_psum[:, :], mmcast(lhs), mmcast(rhs),
                            start=(dhf == 0 and h == 0), stop=False,
                            skip_group_check=True,
                        )
                nc.tensor.matmul(
                    out_psum[:, :], mmcast(onesbo_sb[:, 0:16]),
                    mmcast(onesbo_sb[:, 16:16 + E]),
                    start=False, stop=True, skip_group_check=True,
                )
                out_sb = gpool.tile([16, E], F32, tag="out_sb")
                nc.vector.tensor_copy(out_sb[:, :], out_psum[:, :])
                nc.sync.dma_start(out_d[16 * q:16 * q + 16, :], out_sb[:, :])

    nc.compile()
    return nc


def _get_nc():
    if "nc" not in _CACHED:
        _CACHED["nc"] = _build_nc(use_f32r_dots=False)
    return _CACHED["nc"]


def build_in_maps(inp_q, inp_kv, attn_mask, Wq, Wk, Wv, Wo, bo, topk):
    assert int(topk) == 4
    inp_q = np.ascontiguousarray(np.asarray(inp_q, dtype=np.float32))
    inp_kv = np.ascontiguousarray(np.asarray(inp_kv, dtype=np.float32))
    attn_mask = np.asarray(attn_mask)
    Wq = np.asarray(Wq, dtype=np.float32)
    Wk = np.asarray(Wk, dtype=np.float32)
    Wv = np.asarray(Wv, dtype=np.float32)
    Wo = np.asarray(Wo, dtype=np.float32)
    bo = np.asarray(bo, dtype=np.float32)

    scale = DH ** -0.5
    # Fold the query projection and Wk into per-instance score vectors t (tiny).
    q = (inp_q[:, 0, :] @ Wq) * scale                                  # [B, H*DH]
    t = np.einsum("ihc,dhc->ihd", q.reshape(B, H, DH),
                  Wk.reshape(D, H, DH)).astype(np.float32)             # [B, H, D]
    # Fold Wv @ Wo per head (tiny).
    w2 = np.einsum("dhc,hce->hde", Wv.reshape(D, H, DH),
                   Wo.reshape(H, DH, E)).astype(np.float32)            # [H, D, E]

    # tq layout: [128 p=d%128, (blk, dh, g, m)] with m<8 = head, else zero-pad.
    tt = np.zeros((B, 2, 128, 32), np.float32)
    tt[:, :, :, :8] = t.reshape(B, H, 2, 128).transpose(0, 2, 3, 1)
    # w2 layout: [128 p, (h, dh, e)]
    w2_host = np.ascontiguousarray(
        w2.reshape(H, 2, 128, E).transpose(2, 0, 1, 3).reshape(128, H * 2 * E))

    neg = (-10000.0 * (1.0 - attn_mask.astype(np.float32))).astype(np.float32)  # [B, J]

    negind = np.zeros((4, 128), np.float32)
    for g in range(4):
        negind[g, 32 * g:32 * g + 32] = 1.0
    hsm = np.zeros((128, 4, 16), np.float32)
    for pi in range(4):
        for g in range(4):
            hsm[32 * g:32 * g + 8, pi, pi * 4 + g] = 1.0
    hsm = hsm.reshape(128, 64)
    ident = np.eye(128, dtype=np.float32)
    onesbo = np.concatenate([np.ones(16, np.float32), bo]).reshape(1, 16 + E)

    in_maps = []
    for c in range(NCORES):
        sl = slice(c * BL, (c + 1) * BL)
        kv_c = inp_kv[sl]                                              # [BL, J, D]
        # per-instance tiles [128, (jc, d)], then group 4 instances per block
        kv_host = np.ascontiguousarray(
            kv_c.reshape(BL, 4, 128, D).transpose(0, 2, 1, 3)          # [BL,128,4,D]
            .reshape(NBLK, 4, 128, 4 * D).transpose(0, 2, 1, 3)
            .reshape(NBLK, 128, 4 * 4 * D))
        kvt_host = np.ascontiguousarray(
            kv_c.transpose(0, 2, 1).reshape(BL, 2, 128, J)
            .transpose(0, 2, 1, 3)                                     # [BL,128,2,J]
            .reshape(NBLK, 4, 128, 2 * J).transpose(0, 2, 1, 3)
            .reshape(NBLK, 128, 4 * 2 * J))
        tq_host = np.ascontiguousarray(
            tt[sl].reshape(NBLK, 4, 2, 128, 32).transpose(3, 0, 2, 1, 4)
            .reshape(128, NBLK * 2 * 4 * 32))
        neg_host = np.ascontiguousarray(
            neg[sl].reshape(NBLK, 4, J).transpose(1, 0, 2).reshape(4, NBLK * J))
        in_maps.append({
            "tq": tq_host, "w2": w2_host, "neg": neg_host,
            "negind": negind, "hsm": hsm, "ident": ident, "onesbo": onesbo,
            "kv": kv_host, "kvt": kvt_host,
        })
    return in_maps


def kernel(inp_q, inp_kv, attn_mask, Wq, Wk, Wv, Wo, bo, topk):
    in_maps = build_in_maps(inp_q, inp_kv, attn_mask, Wq, Wk, Wv, Wo, bo, topk)
    nc = _get_nc()
    res = run_bass_kernel_spmd(nc, in_maps, list(range(NCORES)))
    global LAST_RESULT
    LAST_RESULT = res

    out = np.empty((B, 1, E), np.float32)
    idx = np.empty((B, 1, 4), np.int32)
    for c in range(NCORES):
        r = res.results[c]
        out[c * BL:(c + 1) * BL, 0, :] = r["out"]
        idx[c * BL:(c + 1) * BL, 0, :] = r["idx"][:, :4].astype(np.int32)
    return out, idx
